# revision 2
# baseline (speedup 1.0000x reference)
"""Trainium2 Bass kernel for nn_CausalFeatureTransformer (v7 schedule).

Label-query attention collapses to a softmax over features (v2 algebra),
rebuilt around chain latency:

 - Z host-cast to fp16, node-contiguous (p t) layout, one t-slice per
   DMA on the two HW queues. fp16 output; host unpermutes and upcasts.
 - LN1 normalization is folded away exactly:
     s = zn*rsqrt(zn^2*vf + eps) == c*rsqrt(c^2*vf + eps*(v+eps)),
   with c = za - mean. The eps*(v+eps) term enters as a rank-1 PE
   broadcast (invvf row x ev row), so no rstd / no Ln+Exp on the LN1
   path and the s-chain starts right after the mean.
 - ACT runs: the s-chain Ln/Exp (vf rides the Ln scale), 4 per-head
   fused exps per chunk (scale=G bias=D), LN2 rstd (Ln+Exp), gelu.
   The gelu table load is forced after the last set-6 op by routing
   the gelu bias through a tile that depends on LN2's last Exp.
 - DVE keeps PSUM readers + copies; all 16-bit operands where legal
   (fp16 everywhere; exp-domain values fit fp16: scores <= 3.4).
 - b2 rides the x_ps init matmul; y drains as a plain fp16 copy.
"""

import math

import numpy as np

D_FEAT, D_EMB, H, DK = 128, 64, 4, 16
SEQ = D_FEAT + 1
N = 4096
N_CORES = 8
NS = N // N_CORES  # 512 nodes per core
NCH, CH, TPC = 2, 256, 2  # chunks per core, nodes per chunk, t per chunk
EPS = 1e-5
EVS = 4096.0  # fp16-subnormal dodge: ev scaled up, invvf scaled down

_CACHE = {}


def _ln64(x, eps=EPS):
    m = x.mean(-1, keepdims=True)
    v = ((x - m) ** 2).mean(-1, keepdims=True)
    return (x - m) / np.sqrt(v + eps)


def _host_consts(A_full, feat_emb, label_token, wq, bq, wk, bk, wv, bv, wo, bo,
                 w1, b1, w2, b2, alpha, g1, be1, g2, be2):
    """Fold all O(params) quantities on the host (float64 for stability)."""
    d = np.float64
    fe = feat_emb.astype(d)
    mu = fe.mean(1, keepdims=True)
    vf = ((fe - mu) ** 2).mean(1)                    # (128,)
    cf = (fe - mu) * g1.astype(d)                    # (128,64)

    t = _ln64(label_token.astype(d)[0, 0]) * g1.astype(d) + be1.astype(d)
    qlab = t @ wq.astype(d) + bq.astype(d)
    klab = t @ wk.astype(d) + bk.astype(d)
    vlab = t @ wv.astype(d) + bv.astype(d)

    Ck = cf @ wk.astype(d)                           # (128,64)
    Cv = cf @ wv.astype(d)                           # (128,64)
    bk_p = be1.astype(d) @ wk.astype(d) + bk.astype(d)
    bv_p = be1.astype(d) @ wv.astype(d) + bv.astype(d)

    al = float(alpha)
    rdk = 1.0 / math.sqrt(DK)
    G = np.zeros((H, D_FEAT), d)
    Dm = np.zeros((H, D_FEAT), d)
    slab = np.zeros(H, d)
    for h in range(H):
        blk = slice(h * DK, (h + 1) * DK)
        G[h] = Ck[:, blk] @ qlab[blk] * rdk
        Dm[h] = qlab[blk] @ bk_p[blk] * rdk + al * A_full[:D_FEAT, D_FEAT].astype(d)
        slab[h] = qlab[blk] @ klab[blk] * rdk + al * A_full[D_FEAT, D_FEAT]
    elab = np.exp(slab)                              # (4,)

    c0 = label_token.astype(d)[0, 0] + bv_p @ wo.astype(d) + bo.astype(d)
    w1p = w1.astype(d) * g2.astype(d)[:, None]       # diag(g2) @ w1
    b1p = be2.astype(d) @ w1.astype(d) + b1.astype(d)

    f32 = np.float32
    fp16 = np.float16

    # Head h -> partition strip 32h..32h+16 (num) / 32h..32h+32 (den).
    nbias = elab[:, None] * (vlab - bv_p).reshape(H, DK)     # (4,16)
    nbrow = np.zeros(128, d)
    dbrow = np.zeros(128, d)
    wo_exp = np.zeros((128, D_EMB), d)
    for h in range(H):
        nbrow[32 * h:32 * h + DK] = nbias[h]
        dbrow[32 * h:32 * h + 32] = elab[h]
        wo_exp[32 * h:32 * h + DK] = wo.astype(d)[h * DK:(h + 1) * DK]
    cv_exp = np.zeros((D_FEAT, D_EMB), d)
    for h in range(H):
        cv_exp[:, h * DK:(h + 1) * DK] = Cv[:, h * DK:(h + 1) * DK]

    # fp32 blob (128, 12): vf | G(4) | D(4) | b1p
    blob_f = np.zeros((128, 12), f32)
    blob_f[:, 0] = vf
    blob_f[:, 1:5] = G.T
    blob_f[:, 5:9] = Dm.T
    blob_f[:, 9] = b1p

    # fp16 blob (128, 448): ident | wo_exp | w1p (rows 0:64) | w2 | cv
    bh = np.zeros((128, 448), fp16)
    bh[:, 0:128] = np.eye(128, dtype=fp16)
    bh[:, 128:192] = wo_exp.astype(fp16)
    bh[:64, 192:320] = w1p.astype(fp16)
    bh[:, 320:384] = w2.astype(fp16)
    bh[:, 384:448] = cv_exp.astype(fp16)

    # fp16 row blob (1, 640): nbrow | dbrow | c0rep(2x) | c0+b2 | invvf/EVS
    brow = np.zeros((1, 640), fp16)
    brow[0, 0:128] = nbrow.astype(fp16)
    brow[0, 128:256] = dbrow.astype(fp16)
    brow[0, 256:320] = c0.astype(fp16)
    brow[0, 320:384] = c0.astype(fp16)
    brow[0, 384:448] = (c0 + b2.astype(d)).astype(fp16)
    brow[0, 512:640] = (1.0 / (vf * EVS)).astype(fp16)

    return {"blob_f": blob_f, "bh": bh, "brow": brow}


def _build_bass():
    import concourse.bacc as bacc
    import concourse.mybir as mybir
    import concourse.tile as tile

    f32 = mybir.dt.float32
    fp16 = mybir.dt.float16
    AF = mybir.ActivationFunctionType
    OP = mybir.AluOpType

    # Restrict Ln/Exp to the one table set containing both, so the
    # act-table-load pass cannot ping-pong between sets.
    import concourse.hw_specs as hw_specs
    _orig_gat = hw_specs.get_activation_tables

    def _gat(arch):
        t = {k: set(v) for k, v in _orig_gat(arch).items()}
        for name, funcs in t.items():
            if name != "natural_log_exp_and_others":
                funcs.discard(AF.Exp)
                funcs.discard(AF.Ln)
        return t

    bacc.get_activation_tables = _gat

    nc = bacc.Bacc("TRN2", target_bir_lowering=False, debug=False,
                   num_devices=N_CORES)

    zs = nc.dram_tensor("zs", (NS, D_FEAT), fp16, kind="ExternalInput")
    blob_f_d = nc.dram_tensor("blob_f", (128, 12), f32, kind="ExternalInput")
    bh_d = nc.dram_tensor("bh", (128, 448), fp16, kind="ExternalInput")
    brow_d = nc.dram_tensor("brow", (1, 640), fp16, kind="ExternalInput")
    yt = nc.dram_tensor("yt", (D_EMB, NS), fp16, kind="ExternalOutput")

    with tile.TileContext(nc) as tc:
        with (
            tc.tile_pool(name="cp", bufs=1) as cp,
            tc.tile_pool(name="wk", bufs=1) as wkp,
            tc.tile_pool(name="sm", bufs=2) as sm,
            tc.tile_pool(name="ps", bufs=1, space="PSUM") as ps,
        ):
            # --- input DMAs.  sync HW queue: za t0, t1, bh, brow.
            #     scalar HW queue: za t2, t3.  pool SW queue: blob_f.
            za = wkp.tile([128, 2 * TPC, D_FEAT], fp16, tag="za")
            zre = zs.rearrange("(p t) f -> p (t f)", p=128)
            nc.sync.dma_start(out=za[:, 0, :], in_=zre[:, 0:128])
            nc.scalar.dma_start(out=za[:, 2, :], in_=zre[:, 256:384])
            nc.sync.dma_start(out=za[:, 1, :], in_=zre[:, 128:256])
            nc.scalar.dma_start(out=za[:, 3, :], in_=zre[:, 384:512])
            bh_s = cp.tile([128, 448], fp16, tag="bh", name="bh")
            nc.sync.dma_start(out=bh_s, in_=bh_d[:])
            brow_s = cp.tile([1, 640], fp16, tag="brow", name="brow")
            nc.sync.dma_start(out=brow_s, in_=brow_d[:])
            bf = cp.tile([128, 12], f32, tag="bf", name="bf")
            nc.gpsimd.dma_start(out=bf, in_=blob_f_d[:])

            vfcol = bf[:, 0:1]
            gcol = bf[:, 1:5]
            dcol = bf[:, 5:9]
            b1p = bf[:, 9:10]
            identf = bh_s[:, 0:128]
            wo_m = bh_s[:, 128:192]
            w1p = bh_s[0:64, 192:320]
            w2m = bh_s[:, 320:384]
            cv = bh_s[:, 384:448]
            nbrow = brow_s[0:1, 0:128]
            dbrow = brow_s[0:1, 128:256]
            c0rep = brow_s[0:1, 256:384]
            c0b2 = brow_s[0:1, 384:448]
            invvf = brow_s[0:1, 512:640]

            # eps tile + dummy act to preload the exp/ln act table at t=0
            eps_t = cp.tile([128, 1], f32, tag="eps_t")
            nc.vector.memset(eps_t, EPS)
            dum = cp.tile([1, 1], f32, tag="dum")
            nc.scalar.activation(out=dum, in_=eps_t[0:1, 0:1], func=AF.Exp)

            ones_row = cp.tile([1, CH], fp16, tag="ones_row")
            nc.vector.memset(ones_row, 1.0)
            ones_col = cp.tile([128, 1], f32, tag="ones_col")
            nc.vector.memset(ones_col, 1.0)
            u32 = mybir.dt.uint32
            magic2 = cp.tile([128, TPC], u32, tag="magic2")
            nc.vector.memset(magic2, 0x5f3759df)
            shift1 = cp.tile([128, 1], u32, tag="shift1")
            nc.vector.memset(shift1, 1)
            ones32 = cp.tile([128, 32], fp16, tag="ones32")
            nc.vector.memset(ones32, 1.0)
            onesr = cp.tile([1, 128], fp16, tag="onesr")
            nc.vector.memset(onesr, 1.0)

            C = range(NCH)
            st = {}

            # ---- LN1 stats (DVE) + c = za - mean (DVE) + ev row prep
            for c in C:
                mv = sm.tile([128, TPC, 2], f32, tag=f"mv{c}", bufs=1,
                             name="mv")
                st[c, "mv"] = mv
                for t in range(TPC):
                    st6 = sm.tile([128, 6], f32, tag="st6", name="st6")
                    nc.vector.bn_stats(out=st6, in_=za[:, TPC * c + t, :])
                    nc.vector.bn_aggr(out=mv[:, t, :], in_=st6)
                # ev = EVS*eps*(v+eps), fp16 (scaled to dodge subnormals)
                ev = sm.tile([128, TPC], fp16, tag=f"ev{c}", bufs=1, name="ev")
                nc.vector.tensor_scalar(
                    out=ev, in0=mv[:, :, 1], scalar1=EPS, scalar2=EVS * EPS,
                    op0=OP.add, op1=OP.mult)
                st[c, "ev"] = ev
                cm = sm.tile([128, TPC, D_FEAT], fp16, tag=f"cm{c}", bufs=1,
                             name="cm")
                st[c, "cm"] = cm
                for t in range(TPC):
                    nc.vector.tensor_scalar(
                        out=cm[:, t, :], in0=za[:, TPC * c + t, :],
                        scalar1=mv[:, t, 0:1], scalar2=None, op0=OP.subtract)

            # ---- PE: cT transposes + ev row + epsB broadcast
            for c in C:
                cT = ps.tile([128, CH], fp16, tag=f"A{c}", name="cT")
                st[c, "cT"] = cT
                for t in range(TPC):
                    nc.tensor.transpose(cT[:, t * 128:(t + 1) * 128],
                                        st[c, "cm"][:, t, :], identf)
                evT = ps.tile([1, TPC * 128], fp16, tag=f"B{c}", name="evT")
                for t in range(TPC):
                    nc.tensor.transpose(evT[0:1, t * 128:(t + 1) * 128],
                                        st[c, "ev"][:, t:t + 1], identf)
                evr = sm.tile([1, TPC * 128], fp16, tag=f"evr{c}", bufs=1,
                              name="evr")
                nc.scalar.copy(out=evr, in_=evT)
                epsB = ps.tile([128, CH], f32, tag=f"C{c}", name="epsB")
                st[c, "epsB"] = epsB
                for t in range(TPC):
                    nc.tensor.matmul(epsB[:, t * 128:(t + 1) * 128], invvf,
                                     evr[0:1, t * 128:(t + 1) * 128],
                                     start=True, stop=True)

            # ---- s-chain: csq = Square(cT) (ACT, PSUM read), cTs copy
            #      (ACT), q = csq + epsB (DVE), lns/rr (ACT), sT (DVE)
            for c in C:
                csq = wkp.tile([128, CH], fp16, tag=f"csq{c}", name="csq")
                nc.scalar.activation(out=csq, in_=st[c, "cT"],
                                     func=AF.Square)
                cTs = wkp.tile([128, CH], fp16, tag=f"cTs{c}", name="cTs")
                nc.scalar.copy(out=cTs, in_=st[c, "cT"])
                q = wkp.tile([128, CH], fp16, tag=f"q{c}", name="q")
                nc.vector.tensor_add(out=q, in0=csq, in1=st[c, "epsB"])
                lns = wkp.tile([128, CH], f32, tag=f"lns{c}", name="lns")
                st[c, "lns"] = lns
                nc.scalar.activation(out=lns, in_=q, func=AF.Ln,
                                     scale=vfcol)
                rr = wkp.tile([128, CH], fp16, tag=f"rr{c}", name="rr")
                nc.scalar.activation(out=rr, in_=lns, func=AF.Exp, scale=-0.5)
                sT = wkp.tile([128, CH], fp16, tag=f"sT{c}", name="sT")
                nc.vector.tensor_mul(out=sT, in0=cTs, in1=rr)
                st[c, "sT"] = sT

            # ---- init matmuls (after epsB frees the C tag)
            for c in C:
                den_ps = ps.tile([128, CH], f32, tag=f"B{c}", name="den_ps")
                num_ps = ps.tile([128, CH], f32, tag=f"C{c}", name="num_ps")
                st[c, "den"] = den_ps
                st[c, "num"] = num_ps
                nc.tensor.matmul(den_ps, dbrow, ones_row, start=True,
                                 stop=False)
                nc.tensor.matmul(num_ps, nbrow, ones_row, start=True,
                                 stop=False)

            # ---- per-head fused exp (ACT) + esh (DVE) + matmuls (PE)
            for c in C:
                sT = st[c, "sT"]
                eh = wkp.tile([128, H, CH], fp16, tag=f"eh{c}", name="eh")
                esh = wkp.tile([128, H, CH], fp16, tag=f"esh{c}", name="esh")
                st[c, "eh"] = eh
                for h in range(H):
                    nc.scalar.activation(out=eh[:, h, :], in_=sT, func=AF.Exp,
                                         scale=gcol[:, h:h + 1],
                                         bias=dcol[:, h:h + 1])
                    nc.vector.tensor_mul(out=esh[:, h, :], in0=eh[:, h, :],
                                         in1=sT)
                    nc.tensor.matmul(st[c, "den"][32 * h:32 * h + 32, :],
                                     ones32, eh[:, h, :],
                                     start=False, stop=(h == H - 1),
                                     tile_position=(0, 32 * h))
                    nc.tensor.matmul(st[c, "num"][32 * h:32 * h + DK, :],
                                     cv[:, h * DK:(h + 1) * DK],
                                     esh[:, h, :],
                                     start=False, stop=(h == H - 1),
                                     tile_position=(0, 32 * h))

            # ---- normalize + x in both layouts + LN2 stats
            for c in C:
                rcp = wkp.tile([128, CH], f32, tag=f"rcp{c}", name="rcp")
                nc.vector.reciprocal_approx_fast(out=rcp, in_=st[c, "den"])
                oe = wkp.tile([128, CH], fp16, tag=f"oe{c}", name="oe")
                nc.vector.tensor_mul(out=oe, in0=st[c, "num"], in1=rcp)
                st[c, "oe"] = oe

                x_ps = ps.tile([D_EMB, CH], f32, tag=f"D{c}", name="x_ps")
                st[c, "x"] = x_ps
                nc.tensor.matmul(x_ps, c0b2, ones_row, start=True, stop=False)
                nc.tensor.matmul(x_ps, wo_m, oe, start=False, stop=False)
                xa_ps = ps.tile([128, TPC, D_EMB], f32, tag=f"A{c}",
                                name="xa_ps")
                st[c, "xa"] = xa_ps
                nc.tensor.matmul(xa_ps[:, :, :], onesr, c0rep, start=True,
                                 stop=False)
                for t in range(TPC):
                    nc.tensor.matmul(xa_ps[:, t, :],
                                     oe[:, t * 128:(t + 1) * 128],
                                     wo_m, start=False, stop=True)
                mvb = sm.tile([128, TPC, 2], f32, tag=f"mvb{c}", bufs=1,
                              name="mvb")
                st[c, "mvb"] = mvb
                for t in range(TPC):
                    st6b = sm.tile([128, 6], f32, tag="st6b", name="st6b")
                    nc.vector.bn_stats(out=st6b, in_=xa_ps[:, t, :])
                    nc.vector.bn_aggr(out=mvb[:, t, :], in_=st6b)

            # ---- LN2 rstd (ACT Ln+Exp; set 6) + uh (DVE) + uhT (PE) +
            #      uT copy (DVE)
            for c in C:
                lnvb = sm.tile([128, TPC], f32, tag="lnvb", name="lnvb")
                nc.scalar.activation(out=lnvb, in_=st[c, "mvb"][:, :, 1],
                                     func=AF.Ln, bias=eps_t)
                rstdb = sm.tile([128, TPC], f32, tag=f"rstdb{c}", bufs=1,
                                name="rstdb")
                nc.scalar.activation(out=rstdb, in_=lnvb, func=AF.Exp,
                                     scale=-0.5)
                st[c, "rstdb"] = rstdb
                uT_ps = ps.tile([D_EMB, CH], fp16, tag=f"B{c}", name="uT_ps")
                uT = wkp.tile([D_EMB, CH], fp16, tag=f"uT{c}", name="uT")
                st[c, "uT"] = uT
                for t in range(TPC):
                    uh = sm.tile([128, D_EMB], fp16, tag="uh", name="uh")
                    nc.vector.tensor_scalar(
                        out=uh, in0=st[c, "xa"][:, t, :],
                        scalar1=st[c, "mvb"][:, t, 0:1],
                        scalar2=rstdb[:, t:t + 1],
                        op0=OP.subtract, op1=OP.mult)
                    nc.tensor.transpose(uT_ps[:, t * 128:(t + 1) * 128], uh,
                                        identf)
                    nc.vector.tensor_copy(
                        out=uT[:, t * 128:(t + 1) * 128],
                        in_=uT_ps[:, t * 128:(t + 1) * 128])

            # Gate the gelu-set switch behind the LAST set-6 ACT op (LN2
            # rstd c1 Exp): route the gelu bias through a tile depending
            # on it, so the single table switch happens after all Ln/Exp.
            b1pl = cp.tile([128, 1], f32, tag="b1pl")
            nc.vector.scalar_tensor_tensor(
                out=b1pl, in0=st[1, "rstdb"][:, 0:1], scalar=0.0,
                in1=b1p, op0=OP.mult, op1=OP.add)
            dumg = cp.tile([1, 1], f32, tag="dumg")
            nc.scalar.activation(out=dumg, in_=b1pl[0:1, 0:1], func=AF.Gelu)

            # ---- FFN: h mm per t (PE) -> gelu (ACT) -> w2 (PE) -> y (DVE)
            for c in C:
                h_ps = ps.tile([2 * D_EMB, CH], f32, tag=f"A{c}", name="h_ps")
                hh = wkp.tile([2 * D_EMB, CH], fp16, tag=f"hh{c}", name="hh")
                y_sb = wkp.tile([D_EMB, CH], fp16, tag=f"y{c}", name="y_sb")
                for t in range(TPC):
                    sl = slice(t * 128, (t + 1) * 128)
                    nc.tensor.matmul(h_ps[:, sl], w1p, st[c, "uT"][:, sl],
                                     start=True, stop=True)
                    nc.scalar.activation(out=hh[:, sl], in_=h_ps[:, sl],
                                         func=AF.Gelu, bias=b1pl)
                    nc.tensor.matmul(st[c, "x"][:, sl], w2m, hh[:, sl],
                                     start=False, stop=True)
                    nc.vector.tensor_copy(out=y_sb[:, sl],
                                          in_=st[c, "x"][:, sl])
                nc.sync.dma_start(out=yt[:, c * CH:(c + 1) * CH], in_=y_sb)

    nc.compile()
    return nc


def _get_nc():
    if "nc" not in _CACHE:
        _CACHE["nc"] = _build_bass()
    return _CACHE["nc"]


# node permutation: yt column j (within a core) -> node index
def _perm():
    j = np.arange(NS)
    chunk = j // 256
    tl = (j % 256) // 128
    p = j % 128
    return 4 * p + 2 * chunk + tl


_PERM = _perm()


def kernel(Z, A_full, feat_emb, label_token, wq, bq, wk, bk, wv, bv, wo, bo,
           w1, b1, w2, b2, alpha, g1, be1, g2, be2, _trace=False,
           _trace_kwargs=None):
    from concourse.bass_utils import run_bass_kernel_spmd

    Z = np.asarray(Z, dtype=np.float32)
    consts = _host_consts(
        np.asarray(A_full), np.asarray(feat_emb), np.asarray(label_token),
        np.asarray(wq), np.asarray(bq), np.asarray(wk), np.asarray(bk),
        np.asarray(wv), np.asarray(bv), np.asarray(wo), np.asarray(bo),
        np.asarray(w1), np.asarray(b1), np.asarray(w2), np.asarray(b2),
        np.asarray(alpha), np.asarray(g1), np.asarray(be1), np.asarray(g2),
        np.asarray(be2))
    consts = {k: np.ascontiguousarray(v) for k, v in consts.items()}

    nc = _get_nc()
    Zh = Z.astype(np.float16)
    in_maps = []
    for c in range(N_CORES):
        m = dict(consts)
        m["zs"] = np.ascontiguousarray(Zh[c * NS:(c + 1) * NS])
        in_maps.append(m)

    kw = {}
    if _trace:
        kw["trace"] = True
        if _trace_kwargs:
            kw.update(_trace_kwargs)
    res = run_bass_kernel_spmd(nc, in_maps, core_ids=list(range(N_CORES)), **kw)

    out = np.empty((N, D_EMB), np.float32)
    for c in range(N_CORES):
        yc = res.results[c]["yt"].astype(np.float32).T  # (NS, 64), perm order
        out[c * NS + _PERM] = yc
    if _trace:
        return out, res
    return out
